# revision 24
# baseline (speedup 1.0000x reference)
"""3x3 median blur (replicate borders) on 8 TRN2 NeuronCores.

Input : input_batch (32, 512, 512, 3) float32
Output: (32, 512, 512, 3) float32, per-channel 3x3 median, edge-replicated.

Strategy
--------
Pure data parallel: 4 whole images per core. Per image:
  * Rows split into top half (0..255) and bottom half (256..511); each
    partition p holds rows {2p-1..2p+2} of both halves, pixel-interleaved
    in the free dim as [t0 t1 t2 b0 b1 b2] per pixel so every 1-pixel
    horizontal shift is 6 fp16 elements (12B, 4B-aligned -> DVE 2x mode).
  * Columns processed in 4 blocks of 128 output pixels (+1 px halo) so
    tiles are small enough for per-tensor double-buffered pools (deep
    cross-block pipelining, no false ring dependencies).
  * Vertical pass: column sort3 (lo, mid, hi); the pairwise min/max over
    rows (2p, 2p+1) is computed once and shared by both output rows
    (5 ops/elem instead of 6).
  * Horizontal pass: med9 = med3(max3(lo), med3(mid), min3(hi)); the
    sliding-window pair max/min for each chain is computed at half
    density (one pair serves the even and odd window that straddle it),
    cutting the total to ~15 min/max ops per element (from 18).
  * All min/max runs on the vector engine in fp16 2x mode (see
    ENGINE_MAP for why gpsimd offload is disabled); f32<->fp16
    conversion + (de)interleave on the scalar engine; DMA on sync
    (HWDGE), with per-tensor double/triple-buffered pools so blocks
    pipeline with ~93% DVE occupancy.

fp16 is safe here: values are in [0, 255), the median is an order
statistic, so the result is an input value rounded to fp16 (rel err
<= ~1e-3, far below any meaningful threshold for this problem).
"""

import numpy as np

import concourse.bass as bass
import concourse.mybir as mybir
from concourse.tile import TileContext
from concourse.vector_clock import ScopedClock
from concourse.bass_utils import run_bass_kernel_spmd

F32 = mybir.dt.float32
F16 = mybir.dt.float16
MIN = mybir.AluOpType.min
MAX = mybir.AluOpType.max

N_CORES = 8
B, H, W, C = 32, 512, 512, 3
WC = W * C                      # 1536 f32 elements per image row
IMGS_PER_CORE = B // N_CORES    # 4
HH = H // 2                     # rows per half (256)
P = 128                         # SBUF partitions
NBLK = 4                        # column blocks per image
BPX = W // NBLK                 # output pixels per block (128)
SPX = BPX + 2                   # stored pixels per block (130, 1px halo)
NG = BPX // 2                   # pair groups per block (64)


class _TileContext(TileContext):
    """TileContext whose final drain splits its semaphore waits.

    The stock TileContext attaches every end-of-kernel semaphore wait to a
    single Drain instruction; walrus' CTRL encoding fits only one sync wait
    per instruction, so kernels touching more than one processor fail to
    compile. Carry the waits on a chain of nops (one wait each) instead.
    """

    def _drain_and_barrier(self, tick_clock, wait_clock):
        carrier = self.nc.sync.nop(nofuse=True, hint="drain_wait_carrier")
        wait_clock.add_sem_waits(
            carrier.ins, ScopedClock({None: tick_clock.global_clock})
        )
        si = carrier.ins.sync_info
        waits = list(si.on_wait) if si and si.on_wait else []
        if len(waits) > 1:
            si.on_wait = waits[:1]
            for k in range(1, len(waits)):
                extra = self.nc.sync.nop(nofuse=True, hint=f"dwc{k}")
                extra.ins.sync_info = mybir.SyncInfo(
                    on_wait=[waits[k]], on_update=[]
                )
        self.nc.sync.drain()
        self.nc.all_engine_barrier()
        popped = self.nc._tile_sem_poison_stack.pop()
        assert popped is self._sem_poison
        self.nc.clear_and_free_semaphores(list(self.sems.allocated().values()))
        self.nc.all_engine_barrier()


def _split_multi_waits(nc):
    """Walrus in this toolchain encodes at most ONE sync wait per instruction.

    Tile attaches every needed semaphore wait directly to the consuming
    instruction; hoist all but the last onto standalone EventSemaphore
    instructions on the same engine immediately before it.
    """
    for f in nc.m.functions:
        for b in f.blocks:
            il = b.instructions
            out, changed = [], False
            for inst in il:
                si = inst.sync_info
                waits = list(si.on_wait) if si is not None and si.on_wait else []
                if len(waits) > 1:
                    changed = True
                    for w in waits[:-1]:
                        ev = mybir.InstEventSemaphore(
                            name=f"EVW-{nc.next_id()}",
                            engine=inst.engine,
                            ins=[],
                            outs=[],
                            sync_info=mybir.SyncInfo(on_wait=[w], on_update=[]),
                        )
                        out.append(ev)
                    si.on_wait = waits[-1:]
                out.append(inst)
            if changed:
                b.instructions = out


def _emit_block(nc, pools, x, y, img, blk):
    """One (image, column-block) pass: 128 output columns x 512 rows."""
    tt = nc.vector.tensor_tensor
    pt = nc.gpsimd.tensor_tensor

    def tl(name, shape, dt=F16):
        return pools[name].tile(shape, dt, tag=name, name=name)

    xi = x[img]                                        # [H, WC]
    # f32 column range loaded for this block (1px halo, clamped at edges)
    cl = max(0, (BPX * blk - 1) * C)
    ch = min(WC, (BPX * blk + SPX - 1) * C)
    npx = (ch - cl) // C                               # pixels loaded (129/130)
    # stored-pixel range this data lands in
    px0 = 1 if blk == 0 else 0
    px1 = SPX - 1 if blk == NBLK - 1 else SPX

    def rows(r0, r1, step=1):                          # [nrows, npx, C] view
        return xi[r0:r1:step, cl:ch].rearrange("h (px c) -> h px c", c=C)

    # ---- DMA in: stag[p, half, slot, px, c]; slot s = row 2p-1+s per half ----
    stag = tl("stag", [P, 2, 4, npx, C], F32)
    top = xi[0:HH, cl:ch].rearrange("(p r) (px c) -> p r px c", r=2, c=C)
    bot = xi[HH:H, cl:ch].rearrange("(p r) (px c) -> p r px c", r=2, c=C)
    nc.sync.dma_start(stag[:, 0, 1:3, :, :], top)                 # rows 2p,2p+1
    nc.sync.dma_start(stag[:, 1, 1:3, :, :], bot)                 # 256+2p(+1)
    nc.sync.dma_start(stag[:, 0, 3, :, :], rows(2, HH + 1, 2))    # row 2p+2
    nc.sync.dma_start(stag[1:P, 0, 0, :, :], rows(1, 2 * P - 2, 2))  # row 2p-1
    nc.sync.dma_start(stag[:, 1, 0, :, :], rows(HH - 1, H - 2, 2))  # 255+2p
    nc.sync.dma_start(stag[0:P - 1, 1, 3, :, :], rows(HH + 2, H - 1, 2))
    # edge clamps (rows -1 and 512 replicate): p=0 via scalar copy (engines
    # can only start at partition 0/32/64/96), p=127 via a tiny DMA
    nc.scalar.copy(stag[0:1, 0, 0:1, :, :], stag[0:1, 0, 1:2, :, :])
    nc.sync.dma_start(stag[P - 1:P, 1, 3, :, :], rows(H - 1, H))

    # ---- cast f32 -> fp16, interleaving the two halves per pixel ----
    X = tl("x", [P, 4, SPX, 2 * C])
    nc.scalar.copy(X[:, :, px0:px1, 0:C], stag[:, 0, :, :, :])
    nc.scalar.copy(X[:, :, px0:px1, C:2 * C], stag[:, 1, :, :, :])
    if blk == 0:
        nc.scalar.copy(X[:, :, 0:1, :], X[:, :, 1:2, :])          # left pad
    elif blk == NBLK - 1:
        nc.scalar.copy(X[:, :, SPX - 1:SPX, :], X[:, :, SPX - 2:SPX - 1, :])

    # ---- vertical pass: column sort3 sharing one pair per row-pair ----
    # Rows 2p and 2p+1 share the pair over X slots (1,2) = rows (2p, 2p+1):
    # row 2p combines with outer slot 0 (row 2p-1), row 2p+1 with slot 3
    # (row 2p+2).  sort3(a,b,c) with p=min(b,c), q=max(b,c):
    #   lo=min(a,p)  hi=max(a,q)  mid=max(p, min(a,q))
    def e(opname):
        return pt if ENGINE_MAP[opname] == "p" else tt

    vpn = tl("vpn", [P, 1, SPX, 6])
    vpx = tl("vpx", [P, 1, SPX, 6])
    e("vpn")(vpn[:], X[:, 1:2], X[:, 2:3], op=MIN)
    e("vpx")(vpx[:], X[:, 1:2], X[:, 2:3], op=MAX)
    lo = tl("lo", [P, 2, SPX, 6])
    tq = tl("tq", [P, 2, SPX, 6])
    hi = tl("hi", [P, 2, SPX, 6])
    mid = tl("mid", [P, 2, SPX, 6])
    Xa = X[:, 0:4:3]                                   # outer slots (0, 3)
    vpn_b = vpn[:].broadcast_to((P, 2, SPX, 6))        # stride-0 row dim
    vpx_b = vpx[:].broadcast_to((P, 2, SPX, 6))
    e("lo")(lo[:], Xa, vpn_b, op=MIN)                  # min3
    e("tq")(tq[:], Xa, vpx_b, op=MIN)
    e("hi")(hi[:], Xa, vpx_b, op=MAX)                  # max3
    e("mid")(mid[:], vpn_b, tq[:], op=MAX)             # med3

    # ---- horizontal pass ----
    # Window k = stored px (k, k+1, k+2). The pair over (2g+1, 2g+2) serves
    # windows 2g (outer elem at px 2g) and 2g+1 (outer at px 2g+3).
    def ev(t):   # even stored px 0..126
        return t[:, :, 0:SPX - 2:2, :]

    def od(t):   # odd stored px 3..129
        return t[:, :, 3:SPX:2, :]

    def pl(t):   # pair left: px 1..127
        return t[:, :, 1:SPX - 1:2, :]

    def pr(t):   # pair right: px 2..128
        return t[:, :, 2:SPX:2, :]

    # A = max3(lo)
    prmax = tl("prmax", [P, 2, NG, 6])
    A = tl("A", [P, 2, BPX, 6])
    e("prmax")(prmax[:], pl(lo), pr(lo), op=MAX)
    e("Ae")(A[:, :, 0:BPX:2, :], prmax[:], ev(lo), op=MAX)
    e("Ao")(A[:, :, 1:BPX:2, :], prmax[:], od(lo), op=MAX)
    # Cm = min3(hi)
    prmin = tl("prmin", [P, 2, NG, 6])
    Cm = tl("C", [P, 2, BPX, 6])
    e("prmin")(prmin[:], pl(hi), pr(hi), op=MIN)
    e("Ce")(Cm[:, :, 0:BPX:2, :], prmin[:], ev(hi), op=MIN)
    e("Co")(Cm[:, :, 1:BPX:2, :], prmin[:], od(hi), op=MIN)
    # Bm = med3(mid)
    pB = tl("pB", [P, 2, NG, 6])
    qB = tl("qB", [P, 2, NG, 6])
    e("pB")(pB[:], pl(mid), pr(mid), op=MIN)
    e("qB")(qB[:], pl(mid), pr(mid), op=MAX)
    tBe = tl("tBe", [P, 2, NG, 6])
    tBo = tl("tBo", [P, 2, NG, 6])
    e("tBe")(tBe[:], qB[:], ev(mid), op=MIN)
    e("tBo")(tBo[:], qB[:], od(mid), op=MIN)
    Bm = tl("B", [P, 2, BPX, 6])
    e("Be")(Bm[:, :, 0:BPX:2, :], pB[:], tBe[:], op=MAX)
    e("Bo")(Bm[:, :, 1:BPX:2, :], pB[:], tBo[:], op=MAX)
    # med3(A, Bm, Cm)
    s_ = tl("s", [P, 2, BPX, 6])
    u_ = tl("u", [P, 2, BPX, 6])
    v_ = tl("v", [P, 2, BPX, 6])
    O = tl("o", [P, 2, BPX, 6])
    e("s")(s_[:], A[:], Bm[:], op=MIN)
    e("u")(u_[:], A[:], Bm[:], op=MAX)
    e("v")(v_[:], u_[:], Cm[:], op=MIN)
    e("O")(O[:], s_[:], v_[:], op=MAX)                 # med9

    # ---- de-interleave cast back to f32 and DMA out ----
    ot = pools["ostag"].tile([P, 2, 2, BPX, C], F32, tag="ostag", name="ostag")
    nc.scalar.copy(ot[:, 0, :, :, :], O[:, :, :, 0:C])
    nc.scalar.copy(ot[:, 1, :, :, :], O[:, :, :, C:2 * C])
    co = BPX * C * blk
    yt = y[img, 0:HH, co:co + BPX * C].rearrange(
        "(p r) (px c) -> p r px c", r=2, c=C
    )
    yb = y[img, HH:H, co:co + BPX * C].rearrange(
        "(p r) (px c) -> p r px c", r=2, c=C
    )
    nc.sync.dma_start(yt[:, :, :, :], ot[:, 0, :, :, :])
    nc.sync.dma_start(yb[:, :, :, :], ot[:, 1, :, :, :])


POOL_BUFS = {
    "stag": 3, "x": 3, "vpn": 2, "vpx": 2, "lo": 3, "tq": 2, "hi": 3,
    "mid": 3, "prmax": 2, "prmin": 2, "pB": 2, "qB": 2, "tBe": 2, "tBo": 2,
    "A": 2, "C": 2, "B": 2, "s": 2, "u": 2, "v": 2, "o": 2, "ostag": 3,
}

# Which engine runs each min/max op: "v" = DVE (vector), "p" = Pool (gpsimd).
# All on DVE: this toolchain's walrus codegen rejects TensorTensor on the
# Pool engine, and the pre-encoded-InstISA workaround (see _convert_pool_tts)
# compiles but the runtime rejects it at execution.  Offloading ~30% of the
# min/max work to gpsimd would be worth ~25% wall-clock if a future
# toolchain accepts either form (simulated 178us vs 235us) — re-test with
# ENGINE_MAP hi/prmax/Ae/Ao/prmin/Ce/Co -> "p" and _convert_pool_tts enabled.
ENGINE_MAP = {
    "vpn": "v", "vpx": "v", "lo": "v", "tq": "v", "hi": "v", "mid": "v",
    "prmax": "v", "Ae": "v", "Ao": "v",
    "prmin": "v", "Ce": "v", "Co": "v",
    "pB": "v", "qB": "v", "tBe": "v", "tBo": "v", "Be": "v", "Bo": "v",
    "s": "v", "u": "v", "v": "v", "O": "v",
}


# ---- Pool-engine tensor_tensor via pre-encoded ISA ----

def _alu_val(nc, op):
    e = nc.isa.get_enum("NEURON_ISA_TPB_ALU_OP")
    return (
        e.NEURON_ISA_TPB_ALU_OP_MAX.value
        if op == MAX
        else e.NEURON_ISA_TPB_ALU_OP_MIN.value
    )


def _mem_pattern(arg, addr_map):
    ap = [list(d) for d in arg.ap]
    free = ap[1:]
    assert len(free) <= 3, f"too many free dims: {ap}"
    esz = mybir.dt.size(arg.dtype)
    base = addr_map[arg.memref] + arg.offset * esz
    steps, nums = [], []
    for st, n in reversed(free):                       # innermost first
        steps.append(int(st))
        nums.append(int(n))
    while len(steps) < 3:
        steps.append(1)
        nums.append(1)
    assert all(-32768 <= st < 32768 for st in steps), steps
    return {
        "start_addr": {"addr_immediate": base},
        "step_elem": steps,
        "num_elem": nums,
    }


def _convert_pool_tts(nc):
    """Replace InstTensorTensor-on-Pool with equivalent pre-encoded InstISA
    (TENSOR_TENSOR_ARITH_OP).  Must run after Tile allocation (physical APs)
    and before _split_multi_waits.  Walrus patches the Tile semaphores into
    the pre-encoded events field."""
    from concourse import bass_isa

    addr_map = {}
    for f in nc.m.functions:
        for alloc in f.allocations:
            if isinstance(alloc, mybir.MemoryLocationSet):
                for ml in alloc.memorylocations:
                    addr_map[ml.name] = ml.addr
    opcode = nc.isa.Opcode.NEURON_ISA_TPB_OPCODE_TENSOR_TENSOR_ARITH_OP
    fp16 = nc.isa.get_enum("NEURON_ISA_TPB_DTYPE").NEURON_ISA_TPB_DTYPE_FP16.value
    n = 0
    for f in nc.m.functions:
        for blk in f.blocks:
            il = blk.instructions
            for i, inst in enumerate(il):
                if (
                    inst.opcode != "TensorTensor"
                    or inst.engine != mybir.EngineType.Pool
                ):
                    continue
                assert mybir.dt.size(inst.ins[0].dtype) == 2
                struct = {
                    "events": {},
                    "in0_in1_dtype": {"dtype_lo": fp16, "dtype_hi": fp16},
                    "out_dtype": fp16,
                    "op": _alu_val(nc, inst.op),
                    "num_active_channels": int(inst.ins[0].ap[0][1]),
                    "src0_mem_pattern": _mem_pattern(inst.ins[0], addr_map),
                    "src1_mem_pattern": _mem_pattern(inst.ins[1], addr_map),
                    "dst_mem_pattern": _mem_pattern(inst.outs[0], addr_map),
                }
                instr_bytes, _ = bass_isa.isa_struct(nc.isa, opcode, struct)
                isa_inst = mybir.InstISA(
                    name=inst.name,
                    isa_opcode=opcode.value,
                    engine=mybir.EngineType.Pool,
                    instr=instr_bytes,
                    op_name="TENSOR_TENSOR",
                    ins=list(inst.ins),
                    outs=list(inst.outs),
                    ant_dict=struct,
                    verify=False,
                    ant_isa_is_sequencer_only=False,
                )
                isa_inst.sync_info = inst.sync_info
                il[i] = isa_inst
                n += 1
            blk.instructions = il
    return n


def build_median_nc(reps=1, n_imgs=IMGS_PER_CORE, split_waits=True):
    nc = bass.Bass("TRN2")
    x = nc.dram_tensor("x", [IMGS_PER_CORE, H, WC], F32, kind="ExternalInput")
    y = nc.dram_tensor("out", [IMGS_PER_CORE, H, WC], F32, kind="ExternalOutput")
    from contextlib import ExitStack

    with _TileContext(nc) as tc, ExitStack() as es:
        pools = {
            name: es.enter_context(tc.tile_pool(name=name, bufs=bufs))
            for name, bufs in POOL_BUFS.items()
        }
        for _ in range(reps):
            for img in range(n_imgs):
                for blk in range(NBLK):
                    _emit_block(nc, pools, x, y, img, blk)
    if split_waits:
        _split_multi_waits(nc)
    return nc


_NC_CACHE = {}


def kernel(input_batch: np.ndarray) -> np.ndarray:
    input_batch = np.asarray(input_batch)
    assert input_batch.shape == (B, H, W, C), input_batch.shape
    xs = np.ascontiguousarray(input_batch.astype(np.float32, copy=False))
    xs = xs.reshape(B, H, WC)
    if "nc" not in _NC_CACHE:
        _NC_CACHE["nc"] = build_median_nc()
    nc = _NC_CACHE["nc"]
    in_maps = [
        {"x": xs[c * IMGS_PER_CORE:(c + 1) * IMGS_PER_CORE]} for c in range(N_CORES)
    ]
    res = run_bass_kernel_spmd(nc, in_maps, core_ids=list(range(N_CORES)))
    out = np.concatenate([res.results[c]["out"] for c in range(N_CORES)], axis=0)
    return out.reshape(B, H, W, C).astype(np.float32, copy=False)


# revision 30
# speedup vs baseline: 1.0099x; 1.0099x over previous
"""3x3 median blur (replicate borders) on 8 TRN2 NeuronCores.

Input : input_batch (32, 512, 512, 3) float32
Output: (32, 512, 512, 3) float32, per-channel 3x3 median, edge-replicated.

Strategy
--------
Pure data parallel: 4 whole images per core. Per image:
  * Rows split into top half (0..255) and bottom half (256..511); each
    partition p holds rows {2p-1..2p+2} of both halves, pixel-interleaved
    in the free dim as [t0 t1 t2 b0 b1 b2] per pixel so every 1-pixel
    horizontal shift is 6 fp16 elements (12B, 4B-aligned -> DVE 2x mode).
  * Columns processed in 4 blocks (widths BLOCK_WIDTHS, 1px halo) so
    tiles are small enough for per-tensor double-buffered pools (deep
    cross-block pipelining, no false ring dependencies); the narrow
    first/last blocks shorten the pipeline fill and drain.
  * Vertical pass: column sort3 (lo, mid, hi); the pairwise min/max over
    rows (2p, 2p+1) is computed once and shared by both output rows
    (5 ops/elem instead of 6).
  * Horizontal pass: med9 = med3(max3(lo), med3(mid), min3(hi)); the
    sliding-window pair max/min for each chain is computed at half
    density (one pair serves the even and odd window that straddle it),
    cutting the total to ~15 min/max ops per element (from 18).
  * All min/max runs on the vector engine in fp16 2x mode (see
    ENGINE_MAP for why gpsimd offload is disabled); f32<->fp16
    conversion + (de)interleave on the scalar engine; DMA on sync
    (HWDGE), with per-tensor double/triple-buffered pools so blocks
    pipeline with ~93% DVE occupancy.

fp16 is safe here: values are in [0, 255), the median is an order
statistic, so the result is an input value rounded to fp16 (rel err
<= ~1e-3, far below any meaningful threshold for this problem).
"""

import numpy as np

import concourse.bass as bass
import concourse.mybir as mybir
from concourse.tile import TileContext
from concourse.vector_clock import ScopedClock
from concourse.bass_utils import run_bass_kernel_spmd

F32 = mybir.dt.float32
F16 = mybir.dt.float16
MIN = mybir.AluOpType.min
MAX = mybir.AluOpType.max

N_CORES = 8
B, H, W, C = 32, 512, 512, 3
WC = W * C                      # 1536 f32 elements per image row
IMGS_PER_CORE = B // N_CORES    # 4
HH = H // 2                     # rows per half (256)
P = 128                         # SBUF partitions
NBLK = 4                        # column blocks per image
BPX = W // NBLK                 # output pixels per block (128)
SPX = BPX + 2                   # stored pixels per block (130, 1px halo)
NG = BPX // 2                   # pair groups per block (64)


class _TileContext(TileContext):
    """TileContext whose final drain splits its semaphore waits.

    The stock TileContext attaches every end-of-kernel semaphore wait to a
    single Drain instruction; walrus' CTRL encoding fits only one sync wait
    per instruction, so kernels touching more than one processor fail to
    compile. Carry the waits on a chain of nops (one wait each) instead.
    """

    def _drain_and_barrier(self, tick_clock, wait_clock):
        carrier = self.nc.sync.nop(nofuse=True, hint="drain_wait_carrier")
        wait_clock.add_sem_waits(
            carrier.ins, ScopedClock({None: tick_clock.global_clock})
        )
        si = carrier.ins.sync_info
        waits = list(si.on_wait) if si and si.on_wait else []
        if len(waits) > 1:
            si.on_wait = waits[:1]
            for k in range(1, len(waits)):
                extra = self.nc.sync.nop(nofuse=True, hint=f"dwc{k}")
                extra.ins.sync_info = mybir.SyncInfo(
                    on_wait=[waits[k]], on_update=[]
                )
        self.nc.sync.drain()
        self.nc.all_engine_barrier()
        popped = self.nc._tile_sem_poison_stack.pop()
        assert popped is self._sem_poison
        self.nc.clear_and_free_semaphores(list(self.sems.allocated().values()))
        self.nc.all_engine_barrier()


def _split_multi_waits(nc):
    """Walrus in this toolchain encodes at most ONE sync wait per instruction.

    Tile attaches every needed semaphore wait directly to the consuming
    instruction; hoist all but the last onto standalone EventSemaphore
    instructions on the same engine immediately before it.
    """
    for f in nc.m.functions:
        for b in f.blocks:
            il = b.instructions
            out, changed = [], False
            for inst in il:
                si = inst.sync_info
                waits = list(si.on_wait) if si is not None and si.on_wait else []
                if len(waits) > 1:
                    changed = True
                    for w in waits[:-1]:
                        ev = mybir.InstEventSemaphore(
                            name=f"EVW-{nc.next_id()}",
                            engine=inst.engine,
                            ins=[],
                            outs=[],
                            sync_info=mybir.SyncInfo(on_wait=[w], on_update=[]),
                        )
                        out.append(ev)
                    si.on_wait = waits[-1:]
                out.append(inst)
            if changed:
                b.instructions = out


def _emit_block(nc, pools, x, y, img, px_lo, bpx):
    """One (image, column-block) pass: bpx output columns x 512 rows."""
    tt = nc.vector.tensor_tensor
    pt = nc.gpsimd.tensor_tensor
    SPX = bpx + 2                                      # stored px (1px halo)
    NG = bpx // 2                                      # pair groups
    BPX = bpx
    first = px_lo == 0
    last = px_lo + bpx == W

    def tl(name, shape, dt=F16):
        return pools[name].tile(shape, dt, tag=name, name=name)

    xi = x[img]                                        # [H, WC]
    # f32 column range loaded for this block (1px halo, clamped at edges)
    cl = max(0, (px_lo - 1) * C)
    ch = min(WC, (px_lo + SPX - 1) * C)
    npx = (ch - cl) // C                               # pixels loaded
    # stored-pixel range this data lands in
    px0 = 1 if first else 0
    px1 = SPX - 1 if last else SPX

    def rows(r0, r1, step=1):                          # [nrows, npx, C] view
        return xi[r0:r1:step, cl:ch].rearrange("h (px c) -> h px c", c=C)

    # ---- DMA in: stag[p, half, slot, px, c]; slot s = row 2p-1+s per half ----
    stag = tl("stag", [P, 2, 4, npx, C], F32)
    top = xi[0:HH, cl:ch].rearrange("(p r) (px c) -> p r px c", r=2, c=C)
    bot = xi[HH:H, cl:ch].rearrange("(p r) (px c) -> p r px c", r=2, c=C)
    nc.sync.dma_start(stag[:, 0, 1:3, :, :], top)                 # rows 2p,2p+1
    nc.sync.dma_start(stag[:, 1, 1:3, :, :], bot)                 # 256+2p(+1)

    # ---- cast f32 -> fp16, interleaving the two halves per pixel ----
    # Split by slot group: the inner slots (1,2) depend only on the two
    # main DMAs above, so the vertical pair ops can start while the
    # halo/outer slots are still in flight.
    X = tl("x", [P, 4, SPX, 2 * C])
    nc.scalar.copy(X[:, 1:3, px0:px1, 0:C], stag[:, 0, 1:3, :, :])
    nc.scalar.copy(X[:, 1:3, px0:px1, C:2 * C], stag[:, 1, 1:3, :, :])

    nc.sync.dma_start(stag[:, 0, 3, :, :], rows(2, HH + 1, 2))    # row 2p+2
    nc.sync.dma_start(stag[1:P, 0, 0, :, :], rows(1, 2 * P - 2, 2))  # row 2p-1
    nc.sync.dma_start(stag[:, 1, 0, :, :], rows(HH - 1, H - 2, 2))  # 255+2p
    nc.sync.dma_start(stag[0:P - 1, 1, 3, :, :], rows(HH + 2, H - 1, 2))
    # edge clamps (rows -1 and 512 replicate): p=0 via scalar copy (engines
    # can only start at partition 0/32/64/96), p=127 via a tiny DMA
    nc.scalar.copy(stag[0:1, 0, 0:1, :, :], stag[0:1, 0, 1:2, :, :])
    nc.sync.dma_start(stag[P - 1:P, 1, 3, :, :], rows(H - 1, H))
    # outer slots cast one at a time: a combined (0,3) strided write would
    # make the byte-range overlap tracker serialize the inner-slot readers
    # (vpn/vpx) behind it
    nc.scalar.copy(X[:, 0:1, px0:px1, 0:C], stag[:, 0, 0:1, :, :])
    nc.scalar.copy(X[:, 0:1, px0:px1, C:2 * C], stag[:, 1, 0:1, :, :])
    nc.scalar.copy(X[:, 3:4, px0:px1, 0:C], stag[:, 0, 3:4, :, :])
    nc.scalar.copy(X[:, 3:4, px0:px1, C:2 * C], stag[:, 1, 3:4, :, :])
    if first:
        nc.scalar.copy(X[:, :, 0:1, :], X[:, :, 1:2, :])          # left pad
    elif last:
        nc.scalar.copy(X[:, :, SPX - 1:SPX, :], X[:, :, SPX - 2:SPX - 1, :])

    # ---- vertical pass: column sort3 sharing one pair per row-pair ----
    # Rows 2p and 2p+1 share the pair over X slots (1,2) = rows (2p, 2p+1):
    # row 2p combines with outer slot 0 (row 2p-1), row 2p+1 with slot 3
    # (row 2p+2).  sort3(a,b,c) with p=min(b,c), q=max(b,c):
    #   lo=min(a,p)  hi=max(a,q)  mid=max(p, min(a,q))
    def e(opname):
        return pt if ENGINE_MAP[opname] == "p" else tt

    vpn = tl("vpn", [P, 1, SPX, 6])
    vpx = tl("vpx", [P, 1, SPX, 6])
    e("vpn")(vpn[:], X[:, 1:2], X[:, 2:3], op=MIN)
    e("vpx")(vpx[:], X[:, 1:2], X[:, 2:3], op=MAX)
    lo = tl("lo", [P, 2, SPX, 6])
    tq = tl("tq", [P, 2, SPX, 6])
    hi = tl("hi", [P, 2, SPX, 6])
    mid = tl("mid", [P, 2, SPX, 6])
    Xa = X[:, 0:4:3]                                   # outer slots (0, 3)
    vpn_b = vpn[:].broadcast_to((P, 2, SPX, 6))        # stride-0 row dim
    vpx_b = vpx[:].broadcast_to((P, 2, SPX, 6))
    e("lo")(lo[:], Xa, vpn_b, op=MIN)                  # min3
    e("tq")(tq[:], Xa, vpx_b, op=MIN)
    e("hi")(hi[:], Xa, vpx_b, op=MAX)                  # max3
    e("mid")(mid[:], vpn_b, tq[:], op=MAX)             # med3

    # ---- horizontal pass ----
    # Window k = stored px (k, k+1, k+2). The pair over (2g+1, 2g+2) serves
    # windows 2g (outer elem at px 2g) and 2g+1 (outer at px 2g+3).
    def ev(t):   # even stored px 0..126
        return t[:, :, 0:SPX - 2:2, :]

    def od(t):   # odd stored px 3..129
        return t[:, :, 3:SPX:2, :]

    def pl(t):   # pair left: px 1..127
        return t[:, :, 1:SPX - 1:2, :]

    def pr(t):   # pair right: px 2..128
        return t[:, :, 2:SPX:2, :]

    # A = max3(lo)
    prmax = tl("prmax", [P, 2, NG, 6])
    A = tl("A", [P, 2, BPX, 6])
    e("prmax")(prmax[:], pl(lo), pr(lo), op=MAX)
    e("Ae")(A[:, :, 0:BPX:2, :], prmax[:], ev(lo), op=MAX)
    e("Ao")(A[:, :, 1:BPX:2, :], prmax[:], od(lo), op=MAX)
    # Cm = min3(hi)
    prmin = tl("prmin", [P, 2, NG, 6])
    Cm = tl("C", [P, 2, BPX, 6])
    e("prmin")(prmin[:], pl(hi), pr(hi), op=MIN)
    e("Ce")(Cm[:, :, 0:BPX:2, :], prmin[:], ev(hi), op=MIN)
    e("Co")(Cm[:, :, 1:BPX:2, :], prmin[:], od(hi), op=MIN)
    # Bm = med3(mid)
    pB = tl("pB", [P, 2, NG, 6])
    qB = tl("qB", [P, 2, NG, 6])
    e("pB")(pB[:], pl(mid), pr(mid), op=MIN)
    e("qB")(qB[:], pl(mid), pr(mid), op=MAX)
    tBe = tl("tBe", [P, 2, NG, 6])
    tBo = tl("tBo", [P, 2, NG, 6])
    e("tBe")(tBe[:], qB[:], ev(mid), op=MIN)
    e("tBo")(tBo[:], qB[:], od(mid), op=MIN)
    Bm = tl("B", [P, 2, BPX, 6])
    e("Be")(Bm[:, :, 0:BPX:2, :], pB[:], tBe[:], op=MAX)
    e("Bo")(Bm[:, :, 1:BPX:2, :], pB[:], tBo[:], op=MAX)
    # med3(A, Bm, Cm)
    s_ = tl("s", [P, 2, BPX, 6])
    u_ = tl("u", [P, 2, BPX, 6])
    v_ = tl("v", [P, 2, BPX, 6])
    O = tl("o", [P, 2, BPX, 6])
    e("s")(s_[:], A[:], Bm[:], op=MIN)
    e("u")(u_[:], A[:], Bm[:], op=MAX)
    e("v")(v_[:], u_[:], Cm[:], op=MIN)
    e("O")(O[:], s_[:], v_[:], op=MAX)                 # med9

    # ---- de-interleave cast back to f32 and DMA out ----
    ot = pools["ostag"].tile([P, 2, 2, BPX, C], F32, tag="ostag", name="ostag")
    nc.scalar.copy(ot[:, 0, :, :, :], O[:, :, :, 0:C])
    nc.scalar.copy(ot[:, 1, :, :, :], O[:, :, :, C:2 * C])
    co = px_lo * C
    yt = y[img, 0:HH, co:co + BPX * C].rearrange(
        "(p r) (px c) -> p r px c", r=2, c=C
    )
    yb = y[img, HH:H, co:co + BPX * C].rearrange(
        "(p r) (px c) -> p r px c", r=2, c=C
    )
    nc.sync.dma_start(yt[:, :, :, :], ot[:, 0, :, :, :])
    nc.sync.dma_start(yb[:, :, :, :], ot[:, 1, :, :, :])


# Column-block widths per image (sum = 512, even).  The narrow first block
# shortens the pipeline-fill serial chain (DMA+cast+vertical before the DVE
# warms up); the narrow last block shortens the drain tail.
BLOCK_WIDTHS = [64, 160, 160, 128]

POOL_BUFS = {
    "stag": 2, "x": 3, "vpn": 2, "vpx": 2, "lo": 3, "tq": 2, "hi": 3,
    "mid": 3, "prmax": 2, "prmin": 2, "pB": 2, "qB": 2, "tBe": 2, "tBo": 2,
    "A": 2, "C": 2, "B": 2, "s": 2, "u": 2, "v": 2, "o": 2, "ostag": 3,
}

# Which engine runs each min/max op: "v" = DVE (vector), "p" = Pool (gpsimd).
# All on DVE: this toolchain's walrus codegen rejects TensorTensor on the
# Pool engine, and the pre-encoded-InstISA workaround (see _convert_pool_tts)
# compiles but the runtime rejects it at execution.  Offloading ~30% of the
# min/max work to gpsimd would be worth ~25% wall-clock if a future
# toolchain accepts either form (simulated 178us vs 235us) — re-test with
# ENGINE_MAP hi/prmax/Ae/Ao/prmin/Ce/Co -> "p" and _convert_pool_tts enabled.
ENGINE_MAP = {
    "vpn": "v", "vpx": "v", "lo": "v", "tq": "v", "hi": "v", "mid": "v",
    "prmax": "v", "Ae": "v", "Ao": "v",
    "prmin": "v", "Ce": "v", "Co": "v",
    "pB": "v", "qB": "v", "tBe": "v", "tBo": "v", "Be": "v", "Bo": "v",
    "s": "v", "u": "v", "v": "v", "O": "v",
}


# ---- Pool-engine tensor_tensor via pre-encoded ISA ----

def _alu_val(nc, op):
    e = nc.isa.get_enum("NEURON_ISA_TPB_ALU_OP")
    return (
        e.NEURON_ISA_TPB_ALU_OP_MAX.value
        if op == MAX
        else e.NEURON_ISA_TPB_ALU_OP_MIN.value
    )


def _mem_pattern(arg, addr_map):
    ap = [list(d) for d in arg.ap]
    free = ap[1:]
    assert len(free) <= 3, f"too many free dims: {ap}"
    esz = mybir.dt.size(arg.dtype)
    base = addr_map[arg.memref] + arg.offset * esz
    steps, nums = [], []
    for st, n in reversed(free):                       # innermost first
        steps.append(int(st))
        nums.append(int(n))
    while len(steps) < 3:
        steps.append(1)
        nums.append(1)
    assert all(-32768 <= st < 32768 for st in steps), steps
    return {
        "start_addr": {"addr_immediate": base},
        "step_elem": steps,
        "num_elem": nums,
    }


def _convert_pool_tts(nc):
    """Replace InstTensorTensor-on-Pool with equivalent pre-encoded InstISA
    (TENSOR_TENSOR_ARITH_OP).  Must run after Tile allocation (physical APs)
    and before _split_multi_waits.  Walrus patches the Tile semaphores into
    the pre-encoded events field."""
    from concourse import bass_isa

    addr_map = {}
    for f in nc.m.functions:
        for alloc in f.allocations:
            if isinstance(alloc, mybir.MemoryLocationSet):
                for ml in alloc.memorylocations:
                    addr_map[ml.name] = ml.addr
    opcode = nc.isa.Opcode.NEURON_ISA_TPB_OPCODE_TENSOR_TENSOR_ARITH_OP
    fp16 = nc.isa.get_enum("NEURON_ISA_TPB_DTYPE").NEURON_ISA_TPB_DTYPE_FP16.value
    n = 0
    for f in nc.m.functions:
        for blk in f.blocks:
            il = blk.instructions
            for i, inst in enumerate(il):
                if (
                    inst.opcode != "TensorTensor"
                    or inst.engine != mybir.EngineType.Pool
                ):
                    continue
                assert mybir.dt.size(inst.ins[0].dtype) == 2
                struct = {
                    "events": {},
                    "in0_in1_dtype": {"dtype_lo": fp16, "dtype_hi": fp16},
                    "out_dtype": fp16,
                    "op": _alu_val(nc, inst.op),
                    "num_active_channels": int(inst.ins[0].ap[0][1]),
                    "src0_mem_pattern": _mem_pattern(inst.ins[0], addr_map),
                    "src1_mem_pattern": _mem_pattern(inst.ins[1], addr_map),
                    "dst_mem_pattern": _mem_pattern(inst.outs[0], addr_map),
                }
                instr_bytes, _ = bass_isa.isa_struct(nc.isa, opcode, struct)
                isa_inst = mybir.InstISA(
                    name=inst.name,
                    isa_opcode=opcode.value,
                    engine=mybir.EngineType.Pool,
                    instr=instr_bytes,
                    op_name="TENSOR_TENSOR",
                    ins=list(inst.ins),
                    outs=list(inst.outs),
                    ant_dict=struct,
                    verify=False,
                    ant_isa_is_sequencer_only=False,
                )
                isa_inst.sync_info = inst.sync_info
                il[i] = isa_inst
                n += 1
            blk.instructions = il
    return n


def build_median_nc(reps=1, n_imgs=IMGS_PER_CORE, split_waits=True):
    nc = bass.Bass("TRN2")
    x = nc.dram_tensor("x", [IMGS_PER_CORE, H, WC], F32, kind="ExternalInput")
    y = nc.dram_tensor("out", [IMGS_PER_CORE, H, WC], F32, kind="ExternalOutput")
    from contextlib import ExitStack

    with _TileContext(nc) as tc, ExitStack() as es:
        pools = {
            name: es.enter_context(tc.tile_pool(name=name, bufs=bufs))
            for name, bufs in POOL_BUFS.items()
        }
        for _ in range(reps):
            for img in range(n_imgs):
                px_lo = 0
                for bpx in BLOCK_WIDTHS:
                    _emit_block(nc, pools, x, y, img, px_lo, bpx)
                    px_lo += bpx
                assert px_lo == W
    if split_waits:
        _split_multi_waits(nc)
    return nc


_NC_CACHE = {}


def kernel(input_batch: np.ndarray) -> np.ndarray:
    input_batch = np.asarray(input_batch)
    assert input_batch.shape == (B, H, W, C), input_batch.shape
    xs = np.ascontiguousarray(input_batch.astype(np.float32, copy=False))
    xs = xs.reshape(B, H, WC)
    if "nc" not in _NC_CACHE:
        _NC_CACHE["nc"] = build_median_nc()
    nc = _NC_CACHE["nc"]
    in_maps = [
        {"x": xs[c * IMGS_PER_CORE:(c + 1) * IMGS_PER_CORE]} for c in range(N_CORES)
    ]
    res = run_bass_kernel_spmd(nc, in_maps, core_ids=list(range(N_CORES)))
    out = np.concatenate([res.results[c]["out"] for c in range(N_CORES)], axis=0)
    return out.reshape(B, H, W, C).astype(np.float32, copy=False)
